# revision 1
# baseline (speedup 1.0000x reference)
"""DeepSeek-MoE block (B=2, S=2048, D=1024, 16 routed experts top-2, 2 shared)
on 8 Trainium2 NeuronCores.

Strategy:
  - Routing (scores/softmax/top-2) is tiny (~0.13 GFLOP) -> computed on host.
  - Routed experts are computed SPARSELY: only the top-2 experts per token.
    Gates are folded into the gathered token rows (g * u); biases folded in on
    the host, so the device only runs plain matmuls.
  - Expert-parallel: each core owns 2 routed experts (weights resident in
    SBUF). Experts are paired big-count-with-small-count so every core runs
    the same (T_big, T_small) tile counts with minimal padding.
  - The 2 shared experts collapse into one matrix (Ws0+Ws1)/2 -> data-parallel
    over tokens (512 tokens per core).
  - All device matmuls are fp16 x fp16 -> fp32 PSUM (~2.7e-4 rel err).
  - Host applies the final combine: u + scatter(routed) + gate-weighted biases
    + shared + shared bias, in fp32.

Device kernel (per core, SPMD - same NEFF on all 8 cores):
  xr [RT, 128, 1024] fp16: routed token tiles, packed [p, c*128+q] =
     x[tile*128+q, c*128+p] (contraction dim on partitions; 2KB/partition DMA).
  wr [2, 128, 8192] fp16: the core's two expert weights, packed [p, c*1024+o]
     = W[o, c*128+p].
  xs [4, 128, 1024] / ws [128, 8192] fp16: same packing for the shared job.
  yr [RT*128, 1024] fp16, ys [512, 1024] fp16: outputs.
Per 128-token tile: 8 accumulating matmuls (K chunks) x 2 N-halves of 512 into
2 PSUM banks, then DVE/ACT copy-cast fp32->fp16 to SBUF, DMA out via SWDGE.
Input DMAs round-robin both HWDGE rings (sync/scalar); weights load as
per-chunk 256KB tiles so the first matmuls start as soon as chunk 0 lands.
"""

import numpy as np

B, S, D = 2, 2048, 1024
N_R, N_S, TOP_K = 16, 2, 2
N_CORES = 8
EPC = N_R // N_CORES        # experts per core
P = 128                     # partitions / tile rows
NCH = D // P                # contraction chunks (8)
T = B * S                   # tokens (4096)
ST = T // N_CORES // P      # shared token tiles per core (4)

_CACHE = {}                 # (T_big, T_small) -> compiled Bacc


def _build_program(T_big, T_small):
    import concourse.bacc as bacc
    import concourse.mybir as mybir
    import concourse.tile as tile

    f16, f32 = mybir.dt.float16, mybir.dt.float32
    nc = bacc.Bacc("TRN2", target_bir_lowering=False, debug=False)
    RT = T_big + T_small

    xr_d = nc.dram_tensor("xr", [RT, P, NCH * P], f16, kind="ExternalInput")
    wr_d = nc.dram_tensor("wr", [EPC, P, NCH * D], f16, kind="ExternalInput")
    xs_d = nc.dram_tensor("xs", [ST, P, NCH * P], f16, kind="ExternalInput")
    ws_d = nc.dram_tensor("ws", [P, NCH * D], f16, kind="ExternalInput")
    yr_d = nc.dram_tensor("yr", [RT * P, D], f16, kind="ExternalOutput")
    ys_d = nc.dram_tensor("ys", [ST * P, D], f16, kind="ExternalOutput")

    with tile.TileContext(nc) as tc:
        with (
            tc.tile_pool(name="wpool", bufs=1) as wpool,
            # all x tiles resident: a tight bufs count makes a later x-DMA
            # wait on a slot-release sem, stalling the whole HWDGE ring FIFO
            tc.tile_pool(name="xpool", bufs=RT + ST) as xpool,
            tc.tile_pool(name="opool", bufs=6) as opool,
            tc.tile_pool(name="pspool", bufs=4, space="PSUM") as pspool,
        ):
            # input DMAs alternate between the two HWDGE rings
            rr = [nc.sync, nc.scalar]
            rr_i = [0]

            def in_dma(out, in_):
                rr[rr_i[0] % 2].dma_start(out=out, in_=in_)
                rr_i[0] += 1

            # per-chunk weight tiles (256KB each) for fine-grained deps
            def load_w(name, src_row):  # src_row: AP [P, NCH*D]
                tiles = []
                for c in range(NCH):
                    wt = wpool.tile([P, D], f16, tag=f"{name}_{c}")
                    tiles.append(wt)
                return tiles

            w_tiles = {0: load_w("w0", None), 1: load_w("w1", None),
                       2: load_w("ws", None)}

            # (job id, input dram, out dram, #tiles, tile offset, weight src AP)
            jobs = [
                (0, xr_d, yr_d, T_big, 0, wr_d.ap()[0]),
                (1, xr_d, yr_d, T_small, T_big, wr_d.ap()[1]),
                (2, xs_d, ys_d, ST, 0, ws_d.ap()),
            ]

            # Input DMA emission order: x tiles interleaved with weight
            # chunks so no x tile queues behind the whole weight stream.
            # (x_j_t, w chunk) issue order; Tile keeps per-ring FIFO order.
            x_tiles = {}
            x_order = []  # (jid, t) in the order compute consumes them
            for jid, src_d, dst_d, ntiles, toff, _w in jobs:
                for t in range(ntiles):
                    x_order.append((jid, t, src_d, toff))

            def load_x(i):
                jid, t, src_d, toff = x_order[i]
                x = xpool.tile([P, NCH, P], f16, tag="x")
                in_dma(x[:], src_d.ap()[toff + t])
                x_tiles[(jid, t)] = x

            # x0, all w0 chunks, x1-x2, all w1 chunks, x3-x4, ws chunks,
            # then the rest of the x tiles
            load_x(0)
            for c in range(NCH):
                in_dma(w_tiles[0][c][:], jobs[0][5][:, c * D : (c + 1) * D])
            load_x(1); load_x(2)
            for c in range(NCH):
                in_dma(w_tiles[1][c][:], jobs[1][5][:, c * D : (c + 1) * D])
            load_x(3); load_x(4)
            for c in range(NCH):
                in_dma(w_tiles[2][c][:], jobs[2][5][:, c * D : (c + 1) * D])
            for i in range(5, len(x_order)):
                load_x(i)

            # outputs ride SWDGE (gpsimd): its completion sems are separate
            # lanes (DMASW0-7), so compute-paced output DMAs never block the
            # 8 HWDGE lanes that pace the input stream
            out_engines = [nc.gpsimd]
            out_i = [0]
            n_tiles_total = RT + ST

            for jid, src_d, dst_d, ntiles, toff, _wsrc in jobs:
                wt = w_tiles[jid]
                for t in range(ntiles):
                    x = x_tiles[(jid, t)]
                    ps0 = pspool.tile([P, 512], f32, tag="ps0")
                    ps1 = pspool.tile([P, 512], f32, tag="ps1")
                    for c in range(NCH):
                        st, sp = (c == 0), (c == NCH - 1)
                        nc.tensor.matmul(
                            ps0[:], x[:, c, :], wt[c][:, 0:512], start=st, stop=sp
                        )
                        nc.tensor.matmul(
                            ps1[:], x[:, c, :], wt[c][:, 512:1024], start=st, stop=sp
                        )
                    o = opool.tile([P, D], f16, tag="o")
                    row = (toff + t) * P
                    eng = out_engines[out_i[0] % len(out_engines)]
                    out_i[0] += 1
                    if out_i[0] == n_tiles_total:
                        # final tile: copies on both engines (Scalar's DMA-ring
                        # duty is over), ship each half as soon as it lands
                        nc.vector.tensor_copy(o[:, 0:512], ps0[:])
                        nc.scalar.copy(o[:, 512:1024], ps1[:])
                        nc.sync.dma_start(
                            out=dst_d.ap()[row : row + P, 0:512], in_=o[:, 0:512]
                        )
                        nc.scalar.dma_start(
                            out=dst_d.ap()[row : row + P, 512:1024],
                            in_=o[:, 512:1024],
                        )
                    else:
                        # both copies on DVE: the Scalar sequencer doubles as a
                        # DMA-issue ring; a copy queued behind lane-chained DMA
                        # issues lands late and stalls the PE via PSUM reuse
                        nc.vector.tensor_copy(o[:, 0:512], ps0[:])
                        nc.vector.tensor_copy(o[:, 512:1024], ps1[:])
                        eng.dma_start(out=dst_d.ap()[row : row + P, :], in_=o[:])

    nc.compile()
    return nc


def kernel(u, centroids, expert_biases, Wr, br, Ws, bs):
    from concourse.bass_utils import run_bass_kernel_spmd

    out, _ = _run(u, centroids, expert_biases, Wr, br, Ws, bs,
                  run_bass_kernel_spmd, trace=False)
    return out


def _run(u, centroids, expert_biases, Wr, br, Ws, bs, runner, trace=False,
         **runner_kwargs):
    u = np.asarray(u, dtype=np.float32)
    uf = u.reshape(T, D)

    # ---- routing on host (matches jax: softmax with max-subtraction,
    #      top-k ties -> lowest index) ----
    scores = uf @ np.asarray(centroids, np.float32).T
    scores = scores + np.asarray(expert_biases, np.float32)[None, :]
    m = scores.max(axis=1, keepdims=True)
    e = np.exp(scores - m)
    sm = e / e.sum(axis=1, keepdims=True)
    order = np.argsort(-sm, axis=1, kind="stable")[:, :TOP_K]     # [T, 2]
    gates = np.take_along_axis(sm, order, axis=1)                 # [T, 2]

    # ---- dispatch: per-expert contiguous segments, padded to 128;
    #      big experts paired with small ones so tile counts are uniform ----
    flat_e = order.reshape(-1)                                    # [2T]
    tok = np.repeat(np.arange(T), TOP_K)
    gate_f = gates.reshape(-1).astype(np.float32)
    counts = np.bincount(flat_e, minlength=N_R)

    by_count = np.argsort(-counts, kind="stable")                 # desc
    bigs, smalls = by_count[:N_CORES], by_count[N_CORES:][::-1]   # pair i<->i
    T_big = max(int(np.ceil(counts[bigs].max() / P)), 1)
    T_small = max(int(np.ceil(counts[smalls].max() / P)), 1)
    RT = T_big + T_small

    expert_base = np.empty(N_R, np.int64)
    expert_base[bigs] = np.arange(N_CORES) * RT * P
    expert_base[smalls] = np.arange(N_CORES) * RT * P + T_big * P

    sort_o = np.argsort(flat_e, kind="stable")
    starts = np.concatenate([[0], np.cumsum(counts)[:-1]])
    ranks = np.empty(TOP_K * T, np.int64)
    ranks[sort_o] = np.arange(TOP_K * T) - np.repeat(starts, counts)
    pos = expert_base[flat_e] + ranks                             # [2T]

    gx = np.zeros((N_CORES * RT * P, D), np.float32)
    gx[pos] = uf[tok] * gate_f[:, None]
    gx16 = gx.astype(np.float16)

    def pack(x16):  # [R,D] -> [R/128, 128(p), NCH*128], [p, c*128+q]=x[q, c*128+p]
        t = x16.reshape(-1, P, NCH, P)                 # [t, q, c, p]
        return np.ascontiguousarray(t.transpose(0, 3, 2, 1)).reshape(-1, P, NCH * P)

    Ws32 = np.asarray(Ws, np.float32)
    bs32 = np.asarray(bs, np.float32)
    Ws_eff = (Ws32[0] + Ws32[1]) * 0.5
    bs_eff = (bs32[0] + bs32[1]) * 0.5

    def pack_w(w):  # [o,d] -> [128(p), NCH*1024], [p, c*1024+o] = w[o, c*128+p]
        wt = w.T.astype(np.float16).reshape(NCH, P, D)  # [c, p, o]
        return np.ascontiguousarray(wt.transpose(1, 0, 2)).reshape(P, NCH * D)

    ws_packed = pack_w(Ws_eff)
    Wr = np.asarray(Wr, np.float32)
    uf16 = uf.astype(np.float16)

    in_maps = []
    for k in range(N_CORES):
        xr = pack(gx16[k * RT * P : (k + 1) * RT * P])
        wr = np.stack([pack_w(Wr[bigs[k]]), pack_w(Wr[smalls[k]])])
        xs = pack(uf16[k * (T // N_CORES) : (k + 1) * (T // N_CORES)])
        in_maps.append({"xr": xr, "wr": wr, "xs": xs, "ws": ws_packed})

    key = (T_big, T_small)
    if key not in _CACHE:
        _CACHE[key] = _build_program(T_big, T_small)
    nc = _CACHE[key]

    res = runner(nc, in_maps, core_ids=list(range(N_CORES)), trace=trace,
                 **runner_kwargs)

    # ---- host combine ----
    Yr = np.concatenate([r["yr"] for r in res.results]).astype(np.float32)
    Ys = np.concatenate([r["ys"] for r in res.results]).astype(np.float32)
    routed = Yr[pos[0::TOP_K]] + Yr[pos[1::TOP_K]]
    br32 = np.asarray(br, np.float32)
    bias = gates[:, 0, None] * br32[order[:, 0]] + gates[:, 1, None] * br32[order[:, 1]]
    out = uf + routed + bias + Ys + bs_eff[None, :]
    return out.reshape(B, S, D).astype(np.float32), res



# revision 4
# speedup vs baseline: 1.0918x; 1.0918x over previous
"""DeepSeek-MoE block (B=2, S=2048, D=1024, 16 routed experts top-2, 2 shared)
on 8 Trainium2 NeuronCores.

Strategy (v2, hybrid fp8/fp16):
  - Routing (scores/softmax/top-2) on host; top-2 dispatch with per-expert
    contiguous 128-row tiles; experts paired big+small per core (SPMD).
  - Routed expert matmuls use a HYBRID contraction split: the first K8=512
    of the 1024 contraction runs as fp8(e4m3) DoubleRow matmuls (2 K=256
    chunks, 2x PE throughput), the remaining 512 as fp16 (4 K=128 chunks).
    End-to-end rel err ~1.5e-2 (fp8 rounding), under the 2e-2 gate.
    Gates are NOT folded into x (applied on host) so fp8 rows stay in a
    healthy e4m3 range; x/w are pre-scaled by 16 each (exact pow2), so the
    device outputs 256*(u @ W.T); host divides by 256 in the fp32 combine.
  - Shared experts collapse into one fp16 matrix (Ws0+Ws1)/2, data-parallel
    over tokens, full-fp16 accuracy (dominant error budget goes to routed).
  - PE warm-up: a dozen dummy N=256 matmuls on a memset scratch tile run
    during the input-DMA wait window so the HAM clock gate (1.2->2.4 GHz)
    flips before/early-into the real matmul stream.
  - First-matmul latency: the big expert's first fp8 weight chunk and the
    first fp8 x tile are the first transfers on the two HWDGE rings.
  - Tail: the last (shared) tile computes its two PSUM halves back-to-back
    ps0-major; the first half ships over SWDGE while the second computes,
    so the post-stream tail is one 128KB DMA instead of 256KB.
  - No scalar-engine ACTIVATE copies (avoids the 1.3us ACT_TABLE_LOAD in
    the preamble); all PSUM->SBUF copies ride the vector engine.
"""

import numpy as np

B, S, D = 2, 2048, 1024
N_R, N_S, TOP_K = 16, 2, 2
N_CORES = 8
EPC = N_R // N_CORES        # experts per core
P = 128                     # partitions / tile rows
T = B * S                   # tokens (4096)
ST = T // N_CORES // P      # shared token tiles per core (4)

K8 = 512                    # fp8 contraction prefix
NC8 = K8 // 256             # fp8 DoubleRow chunks (K=256 each) -> 2
NC16 = (D - K8) // P        # fp16 chunks for the rest -> 4
NCH = D // P                # full fp16 chunks (shared job) -> 8
QS = 16.0                   # fp8 quant scale for x and w (each)
OSC = QS * QS               # routed output scale (256)
WARM_N = 12                 # warm-up matmuls (N=256 each)

_CACHE = {}                 # (T_big, T_small) -> compiled Bacc


def _build_program(T_big, T_small):
    import concourse.bacc as bacc
    import concourse.mybir as mybir
    import concourse.tile as tile

    f8 = mybir.dt.float8e4
    f16, f32 = mybir.dt.float16, mybir.dt.float32
    DR = mybir.MatmulPerfMode.DoubleRow
    nc = bacc.Bacc("TRN2", target_bir_lowering=False, debug=False)
    RT = T_big + T_small

    xr8_d = nc.dram_tensor("xr8", [RT, P, NC8 * 2 * P], f8, kind="ExternalInput")
    xr16_d = nc.dram_tensor("xr16", [RT, P, NC16 * P], f16, kind="ExternalInput")
    wr8_d = nc.dram_tensor("wr8", [EPC, P, NC8 * 2 * D], f8, kind="ExternalInput")
    wr16_d = nc.dram_tensor("wr16", [EPC, P, NC16 * D], f16, kind="ExternalInput")
    xs_d = nc.dram_tensor("xs", [ST, P, NCH * P], f16, kind="ExternalInput")
    ws_d = nc.dram_tensor("ws", [P, NCH * D], f16, kind="ExternalInput")
    yr_d = nc.dram_tensor("yr", [RT * P, D], f16, kind="ExternalOutput")
    ys_d = nc.dram_tensor("ys", [ST * P, D], f16, kind="ExternalOutput")

    with tile.TileContext(nc) as tc:
        with (
            tc.tile_pool(name="wpool", bufs=1) as wpool,
            # all x tiles resident: a tight bufs count makes a later x-DMA
            # wait on a slot-release sem, stalling the whole HWDGE ring FIFO
            tc.tile_pool(name="xpool", bufs=RT + ST) as xpool,
            tc.tile_pool(name="opool", bufs=6) as opool,
            tc.tile_pool(name="pspool", bufs=3, space="PSUM") as pspool,
            tc.tile_pool(name="warmps", bufs=1, space="PSUM") as warmps,
        ):
            # ---- PE warm-up: dummy matmuls with no DMA dependency ----
            warm = wpool.tile([P, 256], f16, tag="warm")
            nc.gpsimd.memset(warm[:], 0)
            wps = warmps.tile([P, 256], f32, tag="warm")
            for _ in range(WARM_N):
                nc.tensor.matmul(wps[:], warm[:, 0:128], warm[:], start=True, stop=True)

            # ---- input DMAs alternate between the two HWDGE rings ----
            rr = [nc.sync, nc.scalar]
            rr_i = [0]

            def in_dma(out, in_):
                rr[rr_i[0] % 2].dma_start(out=out, in_=in_)
                rr_i[0] += 1

            # SBUF weight tiles (allocated up front; DMA'd in the order below)
            w8_t = {e: [wpool.tile([P, 2, D], f8, tag=f"w8_{e}_{c}", name=f"w8_{e}_{c}") for c in range(NC8)]
                    for e in range(EPC)}
            w16_t = {e: [wpool.tile([P, D], f16, tag=f"w16_{e}_{c}", name=f"w16_{e}_{c}") for c in range(NC16)]
                    for e in range(EPC)}
            ws_t = [wpool.tile([P, D], f16, tag=f"ws_{c}", name=f"ws_{c}") for c in range(NCH)]

            x8_t, x16_t, xs_t = {}, {}, {}

            def load_x(t):
                x8 = xpool.tile([P, NC8, 2, P], f8, tag="x8")
                in_dma(x8[:], xr8_d.ap()[t])
                x16 = xpool.tile([P, NC16, P], f16, tag="x16")
                in_dma(x16[:], xr16_d.ap()[t])
                x8_t[t], x16_t[t] = x8, x16

            def load_xs(t):
                xs = xpool.tile([P, NCH, P], f16, tag="xs")
                in_dma(xs[:], xs_d.ap()[t])
                xs_t[t] = xs

            def load_w8(e, c):
                in_dma(w8_t[e][c][:], wr8_d.ap()[e][:, c * 2 * D:(c + 1) * 2 * D])

            def load_w16(e, c):
                in_dma(w16_t[e][c][:], wr16_d.ap()[e][:, c * D:(c + 1) * D])

            def load_ws(c):
                in_dma(ws_t[c][:], ws_d.ap()[:, c * D:(c + 1) * D])

            # Emission order: the first matmul needs x-tile-0 fp8 + big
            # expert fp8 chunk 0; weights JIT-interleaved with x tiles.
            load_w8(0, 0)                       # sync ring first
            x8 = xpool.tile([P, NC8, 2, P], f8, tag="x8")   # scalar ring
            in_dma(x8[:], xr8_d.ap()[0])
            load_w8(0, 1)
            x16 = xpool.tile([P, NC16, P], f16, tag="x16")
            in_dma(x16[:], xr16_d.ap()[0])
            x8_t[0], x16_t[0] = x8, x16
            for c in range(NC16):
                load_w16(0, c)
            load_x(1)
            load_w8(1, 0); load_w8(1, 1)
            load_x(2)
            for c in range(NC16):
                load_w16(1, c)
            load_x(3)
            load_ws(0); load_ws(1)
            load_x(4)
            load_ws(2); load_ws(3)
            for t in range(5, RT):
                load_x(t)
            load_ws(4); load_ws(5)
            load_xs(0); load_xs(1)
            load_ws(6); load_ws(7)
            load_xs(2); load_xs(3)

            # ---- compute ----
            def routed_tile(e, x8, x16, ps0, ps1):
                for c in range(NC8):
                    st = (c == 0)
                    nc.tensor.matmul(ps0[:], x8[:, c, :, :], w8_t[e][c][:, :, 0:512],
                                     start=st, stop=False, perf_mode=DR)
                    nc.tensor.matmul(ps1[:], x8[:, c, :, :], w8_t[e][c][:, :, 512:1024],
                                     start=st, stop=False, perf_mode=DR)
                for c in range(NC16):
                    sp = (c == NC16 - 1)
                    nc.tensor.matmul(ps0[:], x16[:, c, :], w16_t[e][c][:, 0:512],
                                     start=False, stop=sp)
                    nc.tensor.matmul(ps1[:], x16[:, c, :], w16_t[e][c][:, 512:1024],
                                     start=False, stop=sp)

            def ship(dst_d, row, ps0, ps1):
                o = opool.tile([P, D], f16, tag="o")
                nc.vector.tensor_copy(o[:, 0:512], ps0[:])
                nc.vector.tensor_copy(o[:, 512:1024], ps1[:])
                nc.gpsimd.dma_start(out=dst_d.ap()[row:row + P, :], in_=o[:])

            for e, ntiles, toff in ((0, T_big, 0), (1, T_small, T_big)):
                for t in range(ntiles):
                    ps0 = pspool.tile([P, 512], f32, tag="ps0")
                    ps1 = pspool.tile([P, 512], f32, tag="ps1")
                    routed_tile(e, x8_t[toff + t], x16_t[toff + t], ps0, ps1)
                    ship(yr_d, (toff + t) * P, ps0, ps1)

            for t in range(ST):
                ps0 = pspool.tile([P, 512], f32, tag="ps0")
                ps1 = pspool.tile([P, 512], f32, tag="ps1")
                row = t * P
                if t < ST - 1:
                    for c in range(NCH):
                        st, sp = (c == 0), (c == NCH - 1)
                        nc.tensor.matmul(ps0[:], xs_t[t][:, c, :], ws_t[c][:, 0:512],
                                         start=st, stop=sp)
                        nc.tensor.matmul(ps1[:], xs_t[t][:, c, :], ws_t[c][:, 512:1024],
                                         start=st, stop=sp)
                    ship(ys_d, row, ps0, ps1)
                else:
                    # last tile: ps0-major so its first half ships while the
                    # second half computes; final DMA is only 128KB on the
                    # (by now idle) sync HWDGE ring
                    o = opool.tile([P, D], f16, tag="o")
                    for c in range(NCH):
                        nc.tensor.matmul(ps0[:], xs_t[t][:, c, :], ws_t[c][:, 0:512],
                                         start=(c == 0), stop=(c == NCH - 1))
                    nc.vector.tensor_copy(o[:, 0:512], ps0[:])
                    nc.gpsimd.dma_start(out=ys_d.ap()[row:row + P, 0:512],
                                        in_=o[:, 0:512])
                    for c in range(NCH):
                        nc.tensor.matmul(ps1[:], xs_t[t][:, c, :], ws_t[c][:, 512:1024],
                                         start=(c == 0), stop=(c == NCH - 1))
                    nc.vector.tensor_copy(o[:, 512:1024], ps1[:])
                    nc.sync.dma_start(out=ys_d.ap()[row:row + P, 512:1024],
                                      in_=o[:, 512:1024])

    nc.compile()
    return nc


def kernel(u, centroids, expert_biases, Wr, br, Ws, bs):
    from concourse.bass_utils import run_bass_kernel_spmd

    out, _ = _run(u, centroids, expert_biases, Wr, br, Ws, bs,
                  run_bass_kernel_spmd, trace=False)
    return out


def _run(u, centroids, expert_biases, Wr, br, Ws, bs, runner, trace=False,
         **runner_kwargs):
    import ml_dtypes
    E4 = ml_dtypes.float8_e4m3fn

    u = np.asarray(u, dtype=np.float32)
    uf = u.reshape(T, D)

    # ---- routing on host (matches jax: softmax with max-subtraction,
    #      top-k ties -> lowest index) ----
    scores = uf @ np.asarray(centroids, np.float32).T
    scores = scores + np.asarray(expert_biases, np.float32)[None, :]
    m = scores.max(axis=1, keepdims=True)
    e = np.exp(scores - m)
    sm = e / e.sum(axis=1, keepdims=True)
    order = np.argsort(-sm, axis=1, kind="stable")[:, :TOP_K]     # [T, 2]
    gates = np.take_along_axis(sm, order, axis=1)                 # [T, 2]

    # ---- dispatch: per-expert contiguous segments, padded to 128;
    #      big experts paired with small ones so tile counts are uniform ----
    flat_e = order.reshape(-1)                                    # [2T]
    tok = np.repeat(np.arange(T), TOP_K)
    counts = np.bincount(flat_e, minlength=N_R)

    by_count = np.argsort(-counts, kind="stable")                 # desc
    bigs, smalls = by_count[:N_CORES], by_count[N_CORES:][::-1]   # pair i<->i
    T_big = max(int(np.ceil(counts[bigs].max() / P)), 1)
    T_small = max(int(np.ceil(counts[smalls].max() / P)), 1)
    RT = T_big + T_small

    expert_base = np.empty(N_R, np.int64)
    expert_base[bigs] = np.arange(N_CORES) * RT * P
    expert_base[smalls] = np.arange(N_CORES) * RT * P + T_big * P

    sort_o = np.argsort(flat_e, kind="stable")
    starts = np.concatenate([[0], np.cumsum(counts)[:-1]])
    ranks = np.empty(TOP_K * T, np.int64)
    ranks[sort_o] = np.arange(TOP_K * T) - np.repeat(starts, counts)
    pos = expert_base[flat_e] + ranks                             # [2T]

    # quantized token rows (gates applied on host after the combine)
    uq8 = (uf * QS).astype(E4)                                    # [T, D] e4m3
    u16 = uf.astype(np.float16)

    gx8 = np.zeros((N_CORES * RT * P, K8), E4)
    gx8[pos] = uq8[:, :K8][tok]
    gx16 = np.zeros((N_CORES * RT * P, D - K8), np.float16)
    gx16[pos] = u16[:, K8:][tok]

    def pack8(x):   # [R, K8] f8 -> [R/128, P, NC8*2*P], [p,c,i,q]=x[q, 256c+128i+p]
        t = x.reshape(-1, P, NC8, 2, P)                # [t, q, c, i, p]
        return np.ascontiguousarray(t.transpose(0, 4, 2, 3, 1)).reshape(-1, P, NC8 * 2 * P)

    def pack16(x):  # [R, D-K8] f16 -> [R/128, P, NC16*P], [p,c*128+q]=x[q, 128c+p]
        t = x.reshape(-1, P, NC16, P)                  # [t, q, c, p]
        return np.ascontiguousarray(t.transpose(0, 3, 2, 1)).reshape(-1, P, NC16 * P)

    def pack_full16(x):  # [R, D] f16 -> [R/128, P, NCH*P] (shared job)
        t = x.reshape(-1, P, NCH, P)
        return np.ascontiguousarray(t.transpose(0, 3, 2, 1)).reshape(-1, P, NCH * P)

    def packw8(w):  # [o,d] f32 -> [P, NC8*2*D], [p, c*2D+i*D+o] = e4m3(16w)[o, 256c+128i+p]
        wq = (w[:, :K8] * QS).astype(E4)
        wt = np.ascontiguousarray(wq.T).reshape(NC8, 2, P, D)   # [c, i, p, o]
        return np.ascontiguousarray(wt.transpose(2, 0, 1, 3)).reshape(P, NC8 * 2 * D)

    def packw16(w):  # [o,d] f32 -> [P, NC16*D], [p, c*D+o] = f16(256w)[o, 128c+p]
        wq = (w[:, K8:] * OSC).astype(np.float16)
        wt = np.ascontiguousarray(wq.T).reshape(NC16, P, D)     # [c, p, o]
        return np.ascontiguousarray(wt.transpose(1, 0, 2)).reshape(P, NC16 * D)

    def packws(w):   # shared, fp16 full-D, unscaled
        wt = w.T.astype(np.float16).reshape(NCH, P, D)
        return np.ascontiguousarray(wt.transpose(1, 0, 2)).reshape(P, NCH * D)

    Ws32 = np.asarray(Ws, np.float32)
    bs32 = np.asarray(bs, np.float32)
    Ws_eff = (Ws32[0] + Ws32[1]) * 0.5
    bs_eff = (bs32[0] + bs32[1]) * 0.5
    ws_packed = packws(Ws_eff)
    Wr = np.asarray(Wr, np.float32)

    in_maps = []
    for k in range(N_CORES):
        sl = slice(k * RT * P, (k + 1) * RT * P)
        in_maps.append({
            "xr8": pack8(gx8[sl]),
            "xr16": pack16(gx16[sl]),
            "wr8": np.stack([packw8(Wr[bigs[k]]), packw8(Wr[smalls[k]])]),
            "wr16": np.stack([packw16(Wr[bigs[k]]), packw16(Wr[smalls[k]])]),
            "xs": pack_full16(u16[k * (T // N_CORES):(k + 1) * (T // N_CORES)]),
            "ws": ws_packed,
        })

    key = (T_big, T_small)
    if key not in _CACHE:
        _CACHE[key] = _build_program(T_big, T_small)
    nc = _CACHE[key]

    res = runner(nc, in_maps, core_ids=list(range(N_CORES)), trace=trace,
                 **runner_kwargs)

    # ---- host combine ----
    Yr = np.concatenate([r["yr"] for r in res.results]).astype(np.float32)
    Ys = np.concatenate([r["ys"] for r in res.results]).astype(np.float32)
    g0 = gates[:, 0, None] * (1.0 / OSC)
    g1 = gates[:, 1, None] * (1.0 / OSC)
    routed = g0 * Yr[pos[0::TOP_K]] + g1 * Yr[pos[1::TOP_K]]
    br32 = np.asarray(br, np.float32)
    bias = gates[:, 0, None] * br32[order[:, 0]] + gates[:, 1, None] * br32[order[:, 1]]
    out = uf + routed + bias + Ys + bs_eff[None, :]
    return out.reshape(B, S, D).astype(np.float32), res
